# revision 28
# baseline (speedup 1.0000x reference)
"""TRN2 Bass kernel for the 4-layer encoder-with-reaches model
(nn_EncoderPreTre: B=8, S=512, D=1024, H=16 heads, NL=4 layers).

kernel(**inputs) takes the FULL inputs (src, reaches, emb_table,
qw/kw/vw/ow) and returns the full output tuple (emb, x) matching
reference.reference(). Distribution: data-parallel over the batch --
core b computes batch element b end to end (B == 8 == n_cores); the
embedding-row gather and per-batch contrib vectors are the host-side
sharding step.

v3 structure (per core, residual transposed x^T [1024, 512] in SBUF):
  Every precision-critical matmul runs as a 3-term fp16 hi/lo-pair
  split (Ah@Bh + Ah@Bl + Al@Bh) at 1 PE cycle/row: a fp16 pair carries
  ~22 mantissa bits, and term products accumulate in fp32 PSUM, giving
  fp32-class accuracy at 3 cycles/row (vs 4 for native fp32).
  Weights are pre-scaled x64 on the host so their lo-halves clear the
  fp16 subnormal floor; the scale is removed with exact power-of-2
  multiplies at the PSUM->SBUF split points (q also folds in the 1/8
  softmax scale: unscale 1/512).

  P1: q^T, k^T projections -> fp16 hi/lo pairs (q at 1/512, k at 1/64).
  P2: v^T projection with contrib/64 fused into the PSUM copy
      (vcb = cb (.) v^T, fp32); PE transposes + an ACT copy scaled by
      reaches/contrib produce vp[s, D] = reaches (.) v in fp16.
  P3 heads, software-pipelined with a 1-head skew (head h's softmax
      chain on DVE/ACT overlaps head h-1's transposes/M2 on the PE):
      scores = 3-term fp16 matmul -> row-max -> E = exp(s-m) fp16 with
      fp32 row-sum Z -> P = E*(-c/Z)*mask fp16 -> PE transpose (1
      cyc/row) -> M2[dk,q] = sum_k vp[k,dk]*PT[k,q] fp16.
  P4: a^T = M2-concat + vcb, split into an fp16 pair at 1/64, then a
      single 3-term wo projection (wo x64) updates the residual. The
      v-term rides through a^T, eliminating the OV@x projection.

Numerics: validated host-side vs the fp32 jax reference at ~3.8e-3 max
rel err (gate 2e-2), including exact fp16 rounding/subnormal behavior.
"""
import numpy as np

import concourse.tile as tile
from concourse import bacc, mybir
from concourse.bass_utils import run_bass_kernel_spmd

F32 = mybir.dt.float32
F32R = mybir.dt.float32r
BF16 = mybir.dt.bfloat16
FP16 = mybir.dt.float16
AX = mybir.AxisListType
OP = mybir.AluOpType
AF = mybir.ActivationFunctionType

B, S, D, H, DK, NL = 8, 512, 1024, 16, 64, 4
QC = S // 128
KC = S // 128
DC = D // 128


TRACE = False        # test harness sets True for neuron-profile capture
LAST_RESULT = None   # BassKernelResults of the last kernel() call
_NC_CACHE = {}


def _build(n_layers=NL, n_cores=8):
    nc = bacc.Bacc("TRN2", target_bir_lowering=False, debug=False,
                   num_devices=n_cores)
    d_x0 = nc.dram_tensor("x0t", [D, S], F32, kind="ExternalInput").ap()
    dw = {}
    for nm in ["wq", "wk", "wv", "wo"]:
        dw[nm + "h"] = nc.dram_tensor(nm + "h", [NL, D, D], FP16,
                                      kind="ExternalInput").ap()
        dw[nm + "l"] = nc.dram_tensor(nm + "l", [NL, D, D], FP16,
                                      kind="ExternalInput").ap()
    d_cb = nc.dram_tensor("cb", [128, S], F32, kind="ExternalInput").ap()
    d_rrc = nc.dram_tensor("rrc", [128, KC], F32, kind="ExternalInput").ap()
    d_mask = nc.dram_tensor("maskq", [QC, 128, S], FP16, kind="ExternalInput").ap()
    d_id32 = nc.dram_tensor("ident", [128, 128], F32, kind="ExternalInput").ap()
    d_id16 = nc.dram_tensor("ident16", [128, 128], FP16, kind="ExternalInput").ap()
    d_out = nc.dram_tensor("xt", [D, S], F32, kind="ExternalOutput").ap()

    with tile.TileContext(nc) as tc:
        _emit(nc, tc, n_layers, d_x0, dw,
              d_cb, d_rrc, d_mask, d_id32, d_id16, d_out)
    nc.compile()
    return nc


def _emit(nc, tc, n_layers, d_x0, dw, d_cb, d_rrc, d_mask,
          d_id32, d_id16, d_out):
    ctx_pools = []

    def pool(name, bufs, space="SBUF"):
        p = tc.tile_pool(name=name, bufs=bufs, space=space)
        ctx_pools.append(p)
        return p.__enter__()

    const = pool("const", 1)
    xpool = pool("x", 1)
    actp = pool("act", 1)
    wpool = pool("w", 2)          # 16 tags, double-buffered for prefetch
    epool = pool("E", 8)          # a full head's 4 E tiles live until the
                                  # batched reciprocal; 8 spans two heads
    ppool = pool("P", 4)          # per-t tags; 4 bufs spans the paired skew
    ptpool = pool("PT", 8)
    small = pool("small", 4)
    tmp8 = pool("tmp8", 2)
    psA = pool("psA", 4, "PSUM")
    psT = pool("psT", 2, "PSUM")
    psM = pool("psM", 2, "PSUM")

    cb = const.tile([128, S], F32)
    nc.sync.dma_start(cb[:], d_cb)
    rrc = const.tile([128, KC], F32)
    nc.sync.dma_start(rrc[:], d_rrc)
    id32 = const.tile([128, 128], F32)
    nc.sync.dma_start(id32[:], d_id32)
    id16 = const.tile([128, 128], FP16)
    nc.sync.dma_start(id16[:], d_id16)
    masks = []
    for t in range(QC):
        # maskc: (1 | 1e-6 on the diag) * (-contrib[q]) per q-partition
        mt = const.tile([128, S], FP16, tag=f"mask{t}", name=f"mask{t}")
        nc.sync.dma_start(mt[:], d_mask[t])
        masks.append(mt)

    # residual pair: x ~= xhi + xlo (fp16 each; ~22 mantissa bits total)
    xhi, xlo = [], []
    for c in range(DC):
        xh = xpool.tile([128, S], FP16, tag=f"xh{c}", name=f"xh{c}")
        xl = xpool.tile([128, S], FP16, tag=f"xl{c}", name=f"xl{c}")
        xhi.append(xh)
        xlo.append(xl)
    for c in range(DC):
        xf = tmp8.tile([128, S], F32, tag="xa", name=f"x0f{c}", bufs=2)
        nc.sync.dma_start(xf[:], d_x0[c * 128:(c + 1) * 128, :])
        nc.vector.tensor_copy(xhi[c][:], xf[:])
        nc.vector.tensor_tensor(xlo[c][:], xf[:], xhi[c][:], op=OP.subtract)

    for l in range(n_layers):
        def load_split(nm):
            his, los = [], []
            for ki in range(DC):
                wh = wpool.tile([128, D], FP16, tag=f"w{ki}h", name=f"{nm}h{ki}_{l}")
                nc.sync.dma_start(wh[:], dw[nm + "h"][l, ki * 128:(ki + 1) * 128, :])
                wl = wpool.tile([128, D], FP16, tag=f"w{ki}l", name=f"{nm}l{ki}_{l}")
                nc.sync.dma_start(wl[:], dw[nm + "l"][l, ki * 128:(ki + 1) * 128, :])
                his.append(wh)
                los.append(wl)
            return his, los

        def proj_psums(his, los, outtag, rhs_hi, rhs_lo):
            """3-term-split projection; yields (c, psum_tile) pairs."""
            for c in range(DC):
                p = psA.tile([128, S], F32, tag="psA", name=f"pp{outtag}{c}_{l}")
                n_mm = 3 * DC
                i_mm = 0
                sl = slice(c * 128, (c + 1) * 128)
                for ki in range(DC):
                    for lhsT, rhs in ((his[ki][:, sl], rhs_hi[ki][:]),
                                      (his[ki][:, sl], rhs_lo[ki][:]),
                                      (los[ki][:, sl], rhs_hi[ki][:])):
                        nc.tensor.matmul(
                            p[:], lhsT, rhs, start=(i_mm == 0),
                            stop=(i_mm == n_mm - 1), skip_group_check=True)
                        i_mm += 1
                yield c, p

        def split16(p, hi_t, lo_t, unscale):
            """PSUM -> fp16 hi/lo pair with exact power-of-2 unscale."""
            nc.vector.tensor_scalar(hi_t[:], p[:], unscale, None, op0=OP.mult)
            nc.vector.scalar_tensor_tensor(
                lo_t[:], p[:], unscale, hi_t[:], op0=OP.mult, op1=OP.subtract)

        # ---- q, k projections -> fp16 pairs (q carries the 1/8 scale) ----
        whq, wlq = load_split("wq")
        qhi, qlo = [], []
        for c, p in proj_psums(whq, wlq, "qt", xhi, xlo):
            qh = actp.tile([128, S], FP16, tag=f"qh{c}", name=f"qh{c}_{l}")
            ql = actp.tile([128, S], FP16, tag=f"ql{c}", name=f"ql{c}_{l}")
            split16(p, qh, ql, 1.0 / 512)
            qhi.append(qh)
            qlo.append(ql)
        whk, wlk = load_split("wk")
        khi, klo = [], []
        for c, p in proj_psums(whk, wlk, "kt", xhi, xlo):
            kh = actp.tile([128, S], FP16, tag=f"kh{c}", name=f"kh{c}_{l}")
            kl = actp.tile([128, S], FP16, tag=f"kl{c}", name=f"kl{c}_{l}")
            split16(p, kh, kl, 1.0 / 64)
            khi.append(kh)
            klo.append(kl)

        # ---- v^T projection with cb fused: vcb = (contrib/64) (.) v64^T ----
        whv, wlv = load_split("wv")
        vcb = []
        for c, p in proj_psums(whv, wlv, "vt", xhi, xlo):
            o = actp.tile([128, S], F32, tag=f"vt{c}", name=f"vt{c}_{l}")
            nc.vector.tensor_tensor(o[:], p[:], cb[:], op=OP.mult)
            vcb.append(o)

        # wo weights early: DMA overlaps the whole attention phase
        woh_w, wol_w = load_split("wo")

        # ---- vp[s, D] = reaches (.) v in fp16, via PE transposes ----
        # (emitted inside the head loop to fill the early-head PE bubble)
        vp = [actp.tile([128, D], FP16, tag=f"vp{sc}", name=f"vp{sc}_{l}")
              for sc in range(KC)]

        def emit_vtransposes():
            for sc in range(KC):
                for half in range(2):
                    ps = psA.tile([128, S], F32, tag="psA",
                                  name=f"pv{sc}{half}_{l}")
                    for j in range(4):
                        c = half * 4 + j
                        nc.tensor.matmul(
                            ps[:, j * 128:(j + 1) * 128],
                            vcb[c][:, sc * 128:(sc + 1) * 128], id32[:],
                            is_transpose=True, start=(j == 0), stop=(j == 3),
                            skip_group_check=True)
                    nc.scalar.activation(
                        vp[sc][:, half * 512:(half + 1) * 512], ps[:], AF.Copy,
                        scale=rrc[:, sc:sc + 1])

        # a^T accumulator: per head, af[head rows] = m2 + vcb
        af = [actp.tile([128, S], F32, tag=f"af{c}", name=f"af{c}_{l}")
              for c in range(DC)]

        # ---- attention heads, software-pipelined with a 2-head skew ----
        ahs, als = [None] * DC, [None] * DC

        def emit_scores(h):
            hp = h // 2
            hb = (h % 2) * 64
            negm = small.tile([128, QC], F32, tag="negm", name=f"negm{h}_{l}")
            zst = small.tile([128, QC], F32, tag="zst", name=f"zst{h}_{l}")
            sc_t = small.tile([128, QC], FP16, tag="scl", name=f"scl{h}_{l}")
            escore = []
            for t in range(QC):
                ps = psA.tile([128, S], F32, tag="psA", name=f"sc{h}{t}_{l}")
                tsl = slice(t * 128, (t + 1) * 128)
                hsl = slice(hb, hb + 64)
                i_mm = 0
                for lhsT, rhs in ((qhi[hp][hsl, tsl], khi[hp][hsl, :]),
                                  (qhi[hp][hsl, tsl], klo[hp][hsl, :]),
                                  (qlo[hp][hsl, tsl], khi[hp][hsl, :])):
                    nc.tensor.matmul(ps[:], lhsT, rhs, start=(i_mm == 0),
                                     stop=(i_mm == 2), skip_group_check=True)
                    i_mm += 1
                nc.vector.tensor_reduce(
                    negm[:, t:t + 1], ps[:], axis=AX.X, op=OP.max, negate=True)
                e = epool.tile([128, S], FP16, tag="E", name=f"e{h}{t}_{l}")
                nc.scalar.activation(e[:], ps[:], AF.Exp,
                                     bias=negm[:, t:t + 1], scale=1.0,
                                     accum_out=zst[:, t:t + 1])
                escore.append(e)
            # one batched reciprocal per head; fp16 scalar keeps the STT
            # eligible for the 2-byte fast path (P is 11-bit anyway)
            with nc.allow_low_precision(reason="P path is fp16 (11-bit)"):
                nc.vector.reciprocal(sc_t[:, 0:QC], zst[:, 0:QC])
            Ps = []
            for t in range(QC):
                p = ppool.tile([128, S], FP16, tag=f"P{t}", name=f"p{h}{t}_{l}")
                nc.vector.scalar_tensor_tensor(
                    p[:], escore[t][:], sc_t[:, t:t + 1], masks[t][:],
                    op0=OP.mult, op1=OP.mult)
                Ps.append(p)
            return Ps

        def emit_transposes(h, Ps):
            PTs = []
            for kc in range(KC):
                tp = psT.tile([128, S], FP16, tag="psT", name=f"tp{h}{kc}_{l}")
                for t in range(QC):
                    nc.tensor.matmul(
                        tp[:, t * 128:(t + 1) * 128],
                        Ps[t][:, kc * 128:(kc + 1) * 128], id16[:],
                        is_transpose=True, start=(t == 0), stop=(t == QC - 1),
                        skip_group_check=True)
                pt_sb = ptpool.tile([128, S], FP16, tag="PT", name=f"pt{h}{kc}_{l}")
                if kc == 0:
                    nc.vector.tensor_copy(pt_sb[:], tp[:])
                else:
                    nc.scalar.copy(pt_sb[:], tp[:])
                PTs.append(pt_sb)
            return PTs

        def emit_m2(h, PTs):
            hp = h // 2
            hb = (h % 2) * 64
            m2 = psM.tile([128, S], F32, tag="psM", name=f"m2{h}_{l}")
            off = hb
            for kc in range(KC):
                nc.tensor.matmul(
                    m2[off:off + 64, :], vp[kc][:, h * 64:h * 64 + 64],
                    PTs[kc][:], start=(kc == 0), stop=(kc == KC - 1))
            # af = (m2 + c (.) v^T)/64 in one STT (vcb pre-scaled by 1/64
            # on top of the /64 v-psum unscale; wo weights carry x64)
            nc.vector.scalar_tensor_tensor(
                af[hp][hb:hb + 64, :], m2[off:off + 64, :], 1.0 / 64,
                vcb[hp][hb:hb + 64, :], op0=OP.mult, op1=OP.add)
            if h % 2 == 1:
                # both head-halves of af[hp] done: split into an fp16 pair
                ah = actp.tile([128, S], FP16, tag=f"qh{hp}", name=f"ah{hp}_{l}")
                nc.vector.tensor_copy(ah[:], af[hp][:])
                al = actp.tile([128, S], FP16, tag=f"kh{hp}", name=f"al{hp}_{l}")
                nc.gpsimd.tensor_tensor(al[:], af[hp][:], ah[:], op=OP.subtract)
                ahs[hp] = ah
                als[hp] = al

        def emit_tail(h, Ps):
            emit_m2(h, emit_transposes(h, Ps))

        def emit_tail_pair(item0, item1):
            # both heads' transposes back-to-back, then both M2 groups:
            # longer same-type PE bursts keep the DVFS ramp alive
            PT0 = emit_transposes(item0[0], item0[1])
            PT1 = emit_transposes(item1[0], item1[1])
            emit_m2(item0[0], PT0)
            emit_m2(item1[0], PT1)

        # paired bursts: 24 back-to-back score matmuls (~5us) let the PE
        # DVFS ramp past its ~3us threshold mid-burst; tails run in pairs
        pend = []
        for h in range(H):
            pend.append((h, emit_scores(h)))
            if h == 1:
                emit_vtransposes()
            if h % 2 == 1 and len(pend) > 2:
                emit_tail_pair(pend.pop(0), pend.pop(0))
        for item in pend:
            emit_tail(*item)

        # ---- single wo projection (3-term split) + residual update ----
        for c in range(DC):
            pow_ = psA.tile([128, S], F32, tag="psA", name=f"pow{c}_{l}")
            n_mm = 3 * DC
            i_mm = 0
            sl = slice(c * 128, (c + 1) * 128)
            for ki in range(DC):
                for lhsT, rhs in ((woh_w[ki][:, sl], ahs[ki][:]),
                                  (woh_w[ki][:, sl], als[ki][:]),
                                  (wol_w[ki][:, sl], ahs[ki][:])):
                    nc.tensor.matmul(pow_[:], lhsT, rhs, start=(i_mm == 0),
                                     stop=(i_mm == n_mm - 1),
                                     skip_group_check=True)
                    i_mm += 1
            xa = tmp8.tile([128, S], F32, tag="xa", name=f"xa{c}_{l}", bufs=2)
            nc.gpsimd.tensor_tensor(xa[:], xhi[c][:], xlo[c][:], op=OP.add)
            xb = tmp8.tile([128, S], F32, tag="xb", name=f"xb{c}_{l}", bufs=2)
            nc.vector.tensor_tensor(xb[:], xa[:], pow_[:], op=OP.add)
            if l == n_layers - 1:
                nc.sync.dma_start(d_out[sl, :], xb[:])
            else:
                nc.vector.tensor_copy(xhi[c][:], xb[:])   # fp16 copy rounds
                nc.gpsimd.tensor_tensor(xlo[c][:], xb[:], xhi[c][:],
                                        op=OP.subtract)

    for p in reversed(ctx_pools):
        p.__exit__(None, None, None)


# ---------------- host side ----------------

def _halfpair(w):
    """fp16 hi/lo pair of w (already pre-scaled)."""
    h = w.astype(np.float16)
    lo = (w - h.astype(np.float32)).astype(np.float16)
    return np.ascontiguousarray(h), np.ascontiguousarray(lo)


def _host_prep(src, reaches, emb_table, qw, kw, vw, ow):
    src = np.asarray(src)
    reaches = np.asarray(reaches, dtype=np.float32)
    emb_table = np.asarray(emb_table, dtype=np.float32)
    emb = emb_table[src]
    rs = reaches.sum(-1, keepdims=True)
    contrib = ((rs - reaches) / (rs + 1e-9) * (1.0 - reaches) * 100.0
               ).astype(np.float32)

    qw = np.asarray(qw, np.float32); kw = np.asarray(kw, np.float32)
    vw = np.asarray(vw, np.float32); ow = np.asarray(ow, np.float32)
    # x64 so the fp16 lo-halves clear the subnormal floor; unscaled at
    # the PSUM split points (q additionally folds the 1/8 softmax scale)
    wq = np.ascontiguousarray(np.transpose(qw, (0, 2, 1)) * 64.0)
    wk = np.ascontiguousarray(np.transpose(kw, (0, 2, 1)) * 64.0)
    wv = np.ascontiguousarray(np.transpose(vw, (0, 2, 1)) * 64.0)
    wo = np.ascontiguousarray(np.transpose(ow, (0, 2, 1)) * 64.0)

    wqh, wql = _halfpair(wq)
    wkh, wkl = _halfpair(wk)
    wvh, wvl = _halfpair(wv)
    woh, wol = _halfpair(wo)

    maskbase = np.ones((QC, 128, S), np.float32)
    idx = np.arange(128)
    diagval = np.float32(1.0) - np.float32(0.999999)
    for t in range(QC):
        maskbase[t, idx, t * 128 + idx] = diagval
    ident = np.eye(128, dtype=np.float32)
    ident16 = np.eye(128, dtype=np.float16)

    shared = dict(wqh=wqh, wql=wql, wkh=wkh, wkl=wkl, wvh=wvh, wvl=wvl,
                  woh=woh, wol=wol, ident=ident, ident16=ident16)
    rr_over_c = (reaches / contrib).astype(np.float32)
    in_maps = []
    for b in range(B):
        # fold -contrib[q] (per q-partition) into the mask tiles
        negc_b = -contrib[b].reshape(QC, 128)
        maskq = (maskbase * negc_b[:, :, None]).astype(np.float16)
        in_maps.append(dict(
            shared,
            maskq=maskq,
            x0t=np.ascontiguousarray(emb[b].T),
            # /64 unscales the x64 v-weights; another /64 pre-scales the
            # af accumulation (af = a/64); rrc carries the countering x64
            cb=np.ascontiguousarray(
                np.broadcast_to(contrib[b][None, :] / 4096.0, (128, S))),
            rrc=np.ascontiguousarray(64.0 * rr_over_c[b].reshape(KC, 128).T),
        ))
    return emb, in_maps


def kernel(src, reaches, emb_table, qw, kw, vw, ow):
    global LAST_RESULT
    if "nc" not in _NC_CACHE:
        _NC_CACHE["nc"] = _build(n_layers=NL, n_cores=B)
    nc = _NC_CACHE["nc"]
    emb, in_maps = _host_prep(src, reaches, emb_table, qw, kw, vw, ow)
    res = run_bass_kernel_spmd(nc, in_maps, core_ids=list(range(B)),
                               trace=TRACE)
    LAST_RESULT = res
    x = np.stack([r["xt"].T for r in res.results]).astype(np.float32)
    return emb, x
